# revision 1
# baseline (speedup 1.0000x reference)
"""Trainium2 Bass kernel for C = triu(triu(A) @ triu(B)), N=4096, fp32.

Math: the product of upper-triangular matrices is upper-triangular, so with
host-side triu masking of A and B the kernel output needs no masking: for an
output tile (m, n) (128x128 tile indices), the contraction over k only gets
contributions from k in [m, n]; tiles below the diagonal are exactly zero.

Sharding (8 cores, SPMD, one NEFF): block-cyclic rows. Core j owns the four
128-row tiles {j, 8+j, 16+j, 24+j} of A and C (512 rows per core); B
(triu-masked) is replicated. All cores run the identical program; where the
program's k-range extends past a core's actual triangle the masked A columns
are zero, so the extra matmuls accumulate zeros and stay correct. This makes
the per-core instruction streams (and hence runtimes) identical by
construction - no load imbalance.

Per-core program: the transposed A shard (lhsT layout, [128, 32, 512]) stays
SBUF-resident; B is streamed tile-by-tile ([128, 512], each tile touched
exactly once); C accumulates in PSUM banks (up to 4 live), is copied out via
VectorE and DMAed to DRAM.

Loop structure: for each 512-wide column super-block s (8 of them), for each
k-tile <= 4s+3, load B[k, s-block] once and matmul it against the A tiles of
every owned row-slot t with 8t <= k, accumulating into psum[t].
"""

import os
import sys

for _p in ("/opt/trn_rl_repo", "/root/.axon_site/_ro/trn_rl_repo"):
    if _p not in sys.path:
        sys.path.insert(0, _p)

import numpy as np

N = 4096
P = 128
NCORES = 8
NSLOT = 4  # row-tiles per core
SW = 512  # n super-block width
NS = N // SW  # 8 supers
KT = N // P  # 32 k-tiles

# matmul input dtype: "f32" (exact, 4 cyc/row), "f32r" (tf32-like, 1 cyc/row
# at free dim >= 256), "bf16" (1 cyc/row, half the DMA traffic)
MM_DTYPE = os.environ.get("MM_DTYPE", "bf16")

_cache = {}


def _build(dt_mode):
    import concourse.bacc as bacc
    import concourse.mybir as mybir
    import concourse.tile as tile

    D = {
        "f32": mybir.dt.float32,
        "f32r": mybir.dt.float32r,
        "bf16": mybir.dt.bfloat16,
    }[dt_mode]

    nc = bacc.Bacc(None, target_bir_lowering=False)
    AT = nc.dram_tensor("AT", [P, KT, NSLOT * P], D, kind="ExternalInput")
    # B packed per n-super: B_packed[s, p, ko, w] = triu(B)[128*ko + p, 512*s + w]
    # so a k-chunk load is per-partition contiguous (KCHUNK*512 elements).
    Bm = nc.dram_tensor("B", [NS, P, KT, SW], D, kind="ExternalInput")
    Cm = nc.dram_tensor("C", [NSLOT * P, N], mybir.dt.float32, kind="ExternalOutput")

    KCHUNK = 4
    b_bufs = 12 if dt_mode == "bf16" else 6

    with tile.TileContext(nc) as tc:
        with (
            tc.tile_pool(name="a", bufs=4) as apool,
            tc.tile_pool(name="b", bufs=b_bufs) as bpool,
            tc.tile_pool(name="o", bufs=4) as opool,
            tc.tile_pool(name="ps", bufs=8, space="PSUM") as pspool,
        ):
            # A shard resident in 4 independent tiles so early matmuls only
            # wait on the first chunk
            # A loads go on the Scalar engine's DMA queue so they stream in
            # parallel with the B chunks issued from the Sync queue
            a_tiles = []
            for g in range(4):
                ag = apool.tile([P, 8, NSLOT * P], D, tag=f"a{g}", name="ag")
                nc.scalar.dma_start(ag[:], AT[:, 8 * g : 8 * (g + 1), :])
                a_tiles.append(ag)

            for s in range(NS):
                kmax = 4 * s + 3
                nslots = kmax // 8 + 1
                psums = [
                    pspool.tile([P, SW], mybir.dt.float32, tag="ps", name="ps")
                    for _ in range(nslots)
                ]
                for kc in range(0, kmax + 1, KCHUNK):
                    cnt = min(KCHUNK, kmax + 1 - kc)
                    bt = bpool.tile([P, KCHUNK, SW], D, tag="b", name="bt")
                    nc.sync.dma_start(bt[:, :cnt, :], Bm[s, :, kc : kc + cnt, :])
                    for k in range(kc, kc + cnt):
                        # columns left of 128*(k - 4s) are k < n-tile regions
                        # where triu(B) is zero; skip them
                        w0 = max(0, P * (k - 4 * s))
                        for t in range(k // 8 + 1):
                            nc.tensor.matmul(
                                psums[t][:, w0:SW],
                                a_tiles[k // 8][:, k % 8, P * t : P * (t + 1)],
                                bt[:, k - kc, w0:SW],
                                start=(k == 8 * t),
                                stop=(k == kmax),
                            )
                for t in range(nslots):
                    ot = opool.tile([P, SW], mybir.dt.float32, tag="o", name="ot")
                    nc.vector.tensor_copy(ot[:], psums[t][:])
                    # C stores on the GpSimd queue: keeps the Sync queue free
                    # for B streaming
                    nc.gpsimd.dma_start(
                        Cm[P * t : P * (t + 1), SW * s : SW * (s + 1)], ot[:]
                    )
    nc.compile()
    return nc


def _get_nc():
    if MM_DTYPE not in _cache:
        _cache[MM_DTYPE] = _build(MM_DTYPE)
    return _cache[MM_DTYPE]


def _np_dtype():
    if MM_DTYPE == "bf16":
        import ml_dtypes

        return np.dtype(ml_dtypes.bfloat16)
    return np.dtype(np.float32)


def _make_in_maps(A, B):
    A = np.asarray(A, dtype=np.float32)
    B = np.asarray(B, dtype=np.float32)
    Au = np.triu(A)
    Bu = np.triu(B)

    npdt = _np_dtype()
    # pack: B_packed[s, p, ko, w] = Bu[128*ko + p, 512*s + w]
    Bu_c = np.ascontiguousarray(
        Bu.reshape(KT, P, NS, SW).transpose(2, 1, 0, 3)
    )
    if npdt != np.float32:
        Bu_c = Bu_c.astype(npdt)

    in_maps = []
    for j in range(NCORES):
        rows = np.concatenate(
            [
                np.arange(P * (NCORES * t + j), P * (NCORES * t + j) + P)
                for t in range(NSLOT)
            ]
        )
        A_loc = Au[rows, :]  # [512, 4096]
        # lhsT layout [p, ko, ml]: element = A_loc[ml, ko*128 + p]
        ATd = np.ascontiguousarray(
            A_loc.reshape(NSLOT * P, KT, P).transpose(2, 1, 0)
        )
        if npdt != np.float32:
            ATd = ATd.astype(npdt)
        in_maps.append({"AT": ATd, "B": Bu_c})
    return in_maps


def kernel(A, B):
    from concourse.bass_utils import run_bass_kernel_spmd

    in_maps = _make_in_maps(A, B)
    nc = _get_nc()
    res = run_bass_kernel_spmd(nc, in_maps, core_ids=list(range(NCORES)))

    C = np.zeros((N, N), dtype=np.float32)
    for j in range(NCORES):
        Cj = res.results[j]["C"]
        for t in range(NSLOT):
            m = NCORES * t + j
            C[P * m : P * (m + 1), :] = Cj[P * t : P * (t + 1), :]
    return C



# revision 3
# speedup vs baseline: 1.1562x; 1.1562x over previous
"""Trainium2 Bass kernel for C = triu(triu(A) @ triu(B)), N=4096, fp32.

Math: the product of upper-triangular matrices is upper-triangular, so with
host-side triu masking of A and B the kernel needs no output masking: output
tile (m, n) (128x128 tile indices) only gets contributions from k in [m, n].

Sharding (8 cores, SPMD, one NEFF): 2D grid, 4 row-groups x 2 col-groups.
Core j = (r, c) = (j // 2, j % 2) owns row-tiles {m : m % 4 == r} (8 tiles,
1024 rows) and col-tiles {n : n % 2 == c} (16 tiles, 2048 cols). Versus 1D
row-sharding this (a) cuts per-core HBM traffic ~28MB -> ~16MB because B is
replicated to 4 cores instead of 8, and (b) cuts identical-program masking
waste because the row spread within a slot is 4 instead of 8. All cores run
the identical program; where a core's triangle is smaller than the program's
k-range, the host-side triu masking makes those matmuls accumulate zeros.

Per-core layout: owned cols are packed into 4 local supers of 512 (col-tiles
n = 8s+2i+c, i=0..3, ascending). For super s the program needs k <= 8s+7 and
row slots t with 4t <= 8s+7 (nslots = 2s+2 <= 8 = PSUM bank count).

A is packed k-major, triu only (144 k-major tiles: slot t contributes ks in
[4t, 32)), loaded in 8 chunks on the scalar HW-DGE queue. B is packed per
super with exact triangle trimming (full-width block k < 8s+2 plus narrowing
tails; 272 tiles total), one DMA per super on the sync HW-DGE queue. Both
stay SBUF-resident (13.4MB), so every HBM byte is read exactly once. C is
written in bf16 (psum fp32 -> bf16 copies alternate vector/gpsimd engines)
via the scalar HW-DGE queue, upcast on host.
"""

import sys

for _p in ("/opt/trn_rl_repo", "/root/.axon_site/_ro/trn_rl_repo"):
    if _p not in sys.path:
        sys.path.insert(0, _p)

import numpy as np

N = 4096
P = 128
KT = N // P  # 32 k-tiles
NCORES = 8
RG = 4  # row groups
CG = 2  # col groups
NSLOT = KT // RG  # 8 row-tiles per core
NSUP = 4  # local col supers per core
SW = 512  # super width (cols)

# widths of the tail matmuls (k = 8s+2+j): union over c of cols >= k
TAILW = [384, 384, 256, 256, 128, 128]
TAILOFF = [0, 384, 768, 1024, 1280, 1408]
TAILSZ = 1536

# nslots at k: slots t with 4t <= k (capped at 8)
_NSK = [min(k // 4 + 1, NSLOT) for k in range(KT)]
# k-major packed A offsets (in 128-wide units)
OFFK = [0] * (KT + 1)
for _k in range(KT):
    OFFK[_k + 1] = OFFK[_k] + _NSK[_k]
ATOT = OFFK[KT]  # 144

AKC = 4  # A load chunk: 4 k-tiles per DMA (8 chunks)

SZFULL = [(8 * s + 2) * SW for s in range(NSUP)]
SZSUP = [SZFULL[s] + TAILSZ for s in range(NSUP)]
BOFF = [0] * (NSUP + 1)
for _s in range(NSUP):
    BOFF[_s + 1] = BOFF[_s] + SZSUP[_s]
BTOT = BOFF[NSUP]  # 34816


def _width(s, k):
    """matmul free width at (super s, k): cols the program still covers."""
    if k < 8 * s + 2:
        return SW
    return TAILW[k - (8 * s + 2)]


_cache = {}


def _build():
    import concourse.bacc as bacc
    import concourse.mybir as mybir
    import concourse.tile as tile

    D = mybir.dt.bfloat16
    f32 = mybir.dt.float32

    nc = bacc.Bacc(None, target_bir_lowering=False)
    # A packed k-major, lhsT layout: AT[p, OFFK[k]+t, ml] = Au[(4t+r)*128+ml, k*128+p]
    Am = nc.dram_tensor("AT", [P, ATOT, P], D, kind="ExternalInput")
    # B packed per super (full block + tails), per-partition contiguous
    Bm = nc.dram_tensor("B", [P, BTOT], D, kind="ExternalInput")
    # C rows: slot-major (8*128), cols: super-major (4*512), bf16
    Cm = nc.dram_tensor("C", [NSLOT * P, NSUP * SW], D, kind="ExternalOutput")

    with tile.TileContext(nc) as tc:
        with (
            tc.tile_pool(name="a", bufs=1) as apool,
            tc.tile_pool(name="b", bufs=1) as bpool,
            tc.tile_pool(name="o", bufs=8) as opool,
            tc.tile_pool(name="ps", bufs=8, space="PSUM") as pspool,
        ):
            # A chunks on the scalar HW-DGE queue, in k order (the order the
            # supers consume them)
            a_tiles = []
            for g in range(KT // AKC):
                w = OFFK[AKC * (g + 1)] - OFFK[AKC * g]
                ag = apool.tile([P, w, P], D, tag=f"a{g}", name="ag")
                nc.scalar.dma_start(ag[:], Am[:, OFFK[AKC * g] : OFFK[AKC * (g + 1)], :])
                a_tiles.append(ag)

            # B supers on the sync HW-DGE queue, one DMA each, all resident
            b_tiles = []
            for s in range(NSUP):
                bt = bpool.tile([P, SZSUP[s]], D, tag=f"b{s}", name="bt")
                nc.sync.dma_start(bt[:], Bm[:, BOFF[s] : BOFF[s + 1]])
                b_tiles.append(bt)

            for s in range(NSUP):
                kmax = 8 * s + 7
                ns = 2 * s + 2
                bt = b_tiles[s]
                psums = [
                    pspool.tile([P, SW], f32, tag="ps", name="ps")
                    for _ in range(ns)
                ]
                for k in range(kmax + 1):
                    w = _width(s, k)
                    if k < 8 * s + 2:
                        rhs = bt[:, k * SW : (k + 1) * SW]
                    else:
                        o = SZFULL[s] + TAILOFF[k - (8 * s + 2)]
                        rhs = bt[:, o : o + w]
                    ach = a_tiles[k // AKC]
                    base = OFFK[k] - OFFK[AKC * (k // AKC)]
                    for t in range(min(k // 4 + 1, ns)):
                        nc.tensor.matmul(
                            psums[t][:, SW - w : SW],
                            ach[:, base + t, :],
                            rhs,
                            start=(k == 4 * t),
                            stop=(k == kmax),
                        )
                for t in range(ns):
                    w0 = SW - _width(s, 4 * t)
                    ot = opool.tile([P, SW], D, tag="o", name="ot")
                    # gpsimd(Pool) has no PSUM port; alternate DVE/Activation
                    if t % 2 == 0:
                        nc.vector.tensor_copy(ot[:, w0:SW], psums[t][:, w0:SW])
                    else:
                        nc.scalar.copy(ot[:, w0:SW], psums[t][:, w0:SW])
                    # C stores share the scalar HW-DGE queue (A loads finish
                    # early); the software gpsimd DMA path is ~2x slower
                    nc.scalar.dma_start(
                        Cm[P * t : P * (t + 1), SW * s + w0 : SW * (s + 1)],
                        ot[:, w0:SW],
                    )
    nc.compile()
    return nc


def _get_nc():
    if "nc" not in _cache:
        _cache["nc"] = _build()
    return _cache["nc"]


def _np_bf16():
    import ml_dtypes

    return np.dtype(ml_dtypes.bfloat16)


def _make_in_maps(A, B):
    A = np.asarray(A, dtype=np.float32)
    B = np.asarray(B, dtype=np.float32)
    Au = np.triu(A)
    Bu = np.triu(B)
    bf16 = _np_bf16()

    Au4 = Au.reshape(KT, P, KT, P)  # [mt, ml, kt, p]
    Bu4 = Bu.reshape(KT, P, KT, P)  # [kt, p, nt, q]

    # A payload depends only on r; B payload only on c
    A_r = []
    for r in range(RG):
        ATd = np.empty((P, ATOT, P), dtype=bf16)
        for k in range(KT):
            for t in range(_NSK[k]):
                # lhsT tile: [p, ml] = Au[(4t+r)*128+ml, k*128+p]
                ATd[:, OFFK[k] + t, :] = Au4[4 * t + r, :, k, :].T
        A_r.append(ATd)

    B_c = []
    for c in range(CG):
        segs = []
        for s in range(NSUP):
            nt0 = 8 * s + c
            # full-width block: k < 8s+2, all 4 owned col-tiles of the super
            full = Bu4[: 8 * s + 2, :, nt0 : nt0 + 8 : 2, :]  # [K, p, 4, q]
            segs.append(
                np.ascontiguousarray(full.transpose(1, 0, 2, 3)).reshape(
                    P, SZFULL[s]
                )
            )
            for j, w in enumerate(TAILW):
                k = 8 * s + 2 + j
                i0 = 4 - w // P
                tail = Bu4[k, :, nt0 + 2 * i0 : nt0 + 8 : 2, :]  # [p, 4-i0, q]
                segs.append(np.ascontiguousarray(tail).reshape(P, w))
        B_c.append(np.concatenate(segs, axis=1).astype(bf16))

    in_maps = []
    for j in range(NCORES):
        r, c = j // CG, j % CG
        in_maps.append({"AT": A_r[r], "B": B_c[c]})
    return in_maps


def kernel(A, B):
    from concourse.bass_utils import run_bass_kernel_spmd

    in_maps = _make_in_maps(A, B)
    nc = _get_nc()
    res = run_bass_kernel_spmd(nc, in_maps, core_ids=list(range(NCORES)))

    C4 = np.zeros((KT, P, KT, P), dtype=np.float32)
    for j in range(NCORES):
        r, c = j // CG, j % CG
        Cj = np.asarray(res.results[j]["C"]).astype(np.float32)
        # rows: slot-major (t -> row-tile 4t+r); cols: (s, i) -> col-tile
        # 8s+2i+c, which is exactly c::2 in ascending order
        C4[r::RG, :, c::CG, :] = Cj.reshape(NSLOT, P, KT // CG, P)
    C = C4.reshape(N, N)
    # below-diagonal tiles the program never stores are uninitialized; the
    # triu kills them (diag-tile interiors are exact zeros from the masking)
    return np.triu(C)


# revision 4
# speedup vs baseline: 1.3626x; 1.1785x over previous
"""Trainium2 Bass kernel for C = triu(triu(A) @ triu(B)), N=4096, fp32.

Math: the product of upper-triangular matrices is upper-triangular, so with
host-side triu masking of A and B the kernel needs no output masking: output
tile (m, n) (128x128 tile indices) only gets contributions from k in [m, n].

Sharding (8 cores, SPMD, one NEFF): 2D grid, 4 row-groups x 2 col-groups.
Core j = (r, c) = (j // 2, j % 2) owns row-tiles {m : m % 4 == r} (8 tiles,
1024 rows) and col-tiles {n : n % 2 == c} (16 tiles, 2048 cols). Versus 1D
row-sharding this (a) cuts per-core HBM traffic ~28MB -> ~16MB because B is
replicated to 4 cores instead of 8, and (b) cuts identical-program masking
waste because the row spread within a slot is 4 instead of 8. All cores run
the identical program; where a core's triangle is smaller than the program's
k-range, the host-side triu masking makes those matmuls accumulate zeros.

Per-core layout: owned cols are packed into 4 local supers of 512 (col-tiles
n = 8s+2i+c, i=0..3, ascending). For super s the program needs k <= 8s+7 and
row slots t with 4t <= 8s+7 (nslots = 2s+2 <= 8 = PSUM bank count).

DMA: the 16 DMA engines are one shared pool and pull per-descriptor, so
streams compete by descriptor size; all loads therefore go on a single
HW-DGE queue (sync) in global need-order, interleaving A chunks (k-major
packed triu, 8 k-tiles per chunk) with B chunks (per super: s full-width
[P,8,512] chunks + one last tile holding k=8s,8s+1 plus the 6 narrowing
tails). Everything stays SBUF-resident; each HBM byte is read once.

Compute: per super, phase 1 runs k-major over the full-width chunks (matches
the load order); phase 2 runs t-major over the last 8 ks so slots finish
staggered - each slot's psum->sbuf bf16 copy (DVE; the ACT copy path is ~9x
slower) and C store (scalar HW-DGE queue) issue immediately, pipelining the
drain instead of serializing ~8 copies after the final matmul. C is written
bf16 and upcast on the host.
"""

import sys

for _p in ("/opt/trn_rl_repo", "/root/.axon_site/_ro/trn_rl_repo"):
    if _p not in sys.path:
        sys.path.insert(0, _p)

import numpy as np

N = 4096
P = 128
KT = N // P  # 32 k-tiles
NCORES = 8
RG = 4  # row groups
CG = 2  # col groups
NSLOT = KT // RG  # 8 row-tiles per core
NSUP = 4  # local col supers per core
SW = 512  # super width (cols)

# widths of the tail matmuls (k = 8s+2+j): union over c of cols >= k
TAILW = [384, 384, 256, 256, 128, 128]
TAILOFF = [0, 384, 768, 1024, 1280, 1408]
TAILSZ = 1536

# nslots at k: slots t with 4t <= k (capped at 8)
_NSK = [min(k // 4 + 1, NSLOT) for k in range(KT)]
# k-major packed A offsets (in 128-wide units)
OFFK = [0] * (KT + 1)
for _k in range(KT):
    OFFK[_k + 1] = OFFK[_k] + _NSK[_k]
ATOT = OFFK[KT]  # 144

AKC = 8  # A load chunk: 8 k-tiles per DMA (4 chunks)

# B per super: s full-width chunks of [P, 8, SW] (k in [0, 8s)), then one
# "last" tile [P, 2*SW + TAILSZ] holding k = 8s, 8s+1 full width + 6 tails
LASTSZ = 2 * SW + TAILSZ  # 2560
SZSUP = [8 * s * SW + LASTSZ for s in range(NSUP)]
BOFF = [0] * (NSUP + 1)
for _s in range(NSUP):
    BOFF[_s + 1] = BOFF[_s] + SZSUP[_s]
BTOT = BOFF[NSUP]  # 34816


def _width(s, k):
    """matmul free width at (super s, k): cols the program still covers."""
    if k < 8 * s + 2:
        return SW
    return TAILW[k - (8 * s + 2)]


_cache = {}


def _build():
    import concourse.bacc as bacc
    import concourse.mybir as mybir
    import concourse.tile as tile

    D = mybir.dt.bfloat16
    f32 = mybir.dt.float32

    nc = bacc.Bacc(None, target_bir_lowering=False)
    # A packed k-major, lhsT layout: AT[p, OFFK[k]+t, ml] = Au[(4t+r)*128+ml, k*128+p]
    Am = nc.dram_tensor("AT", [P, ATOT, P], D, kind="ExternalInput")
    # B packed per super (full chunks + last tile), per-partition contiguous
    Bm = nc.dram_tensor("B", [P, BTOT], D, kind="ExternalInput")
    # C rows: slot-major (8*128), cols: super-major (4*512), bf16
    Cm = nc.dram_tensor("C", [NSLOT * P, NSUP * SW], D, kind="ExternalOutput")

    with tile.TileContext(nc) as tc:
        with (
            tc.tile_pool(name="a", bufs=1) as apool,
            tc.tile_pool(name="b", bufs=1) as bpool,
            tc.tile_pool(name="o", bufs=8) as opool,
            tc.tile_pool(name="ps", bufs=8, space="PSUM") as pspool,
        ):
            a_tiles = [None] * (KT // AKC)
            bf8_tiles = [[None] * s for s in range(NSUP)]
            blast_tiles = [None] * NSUP

            def load_a(g):
                w = OFFK[AKC * (g + 1)] - OFFK[AKC * g]
                ag = apool.tile([P, w, P], D, tag=f"a{g}", name="ag")
                nc.sync.dma_start(
                    ag[:], Am[:, OFFK[AKC * g] : OFFK[AKC * (g + 1)], :]
                )
                a_tiles[g] = ag

            def load_bf8(s, i):
                bt = bpool.tile([P, 8, SW], D, tag=f"b{s}f{i}", name="bt")
                o = BOFF[s] + 8 * i * SW
                nc.sync.dma_start(bt[:], Bm[:, o : o + 8 * SW])
                bf8_tiles[s][i] = bt

            def load_blast(s):
                bt = bpool.tile([P, LASTSZ], D, tag=f"b{s}l", name="bl")
                o = BOFF[s] + 8 * s * SW
                nc.sync.dma_start(bt[:], Bm[:, o : o + LASTSZ])
                blast_tiles[s] = bt

            # single queue, global need-order: nothing starves anything
            load_blast(0)
            load_a(0)
            load_bf8(1, 0)
            load_a(1)
            load_blast(1)
            load_bf8(2, 0)
            load_bf8(2, 1)
            load_a(2)
            load_blast(2)
            load_bf8(3, 0)
            load_bf8(3, 1)
            load_bf8(3, 2)
            load_a(3)
            load_blast(3)

            for s in range(NSUP):
                kmax = 8 * s + 7
                ns = 2 * s + 2
                psums = [
                    pspool.tile([P, SW], f32, tag="ps", name="ps")
                    for _ in range(ns)
                ]

                def lhs(k, t):
                    g = k // AKC
                    return a_tiles[g][:, OFFK[k] - OFFK[AKC * g] + t, :]

                # phase 1: k-major over the full-width chunks (load order)
                for k in range(8 * s):
                    rhs = bf8_tiles[s][k // 8][:, k % 8, :]
                    for t in range(k // 4 + 1):
                        nc.tensor.matmul(
                            psums[t][:],
                            lhs(k, t),
                            rhs,
                            start=(k == 4 * t),
                            stop=False,
                        )
                # phase 2: t-major over the last 8 ks; slots finish staggered
                # so the copy+store drain pipelines with remaining matmuls
                bl = blast_tiles[s]
                for t in range(ns):
                    for k in range(max(4 * t, 8 * s), 8 * s + 8):
                        w = _width(s, k)
                        j = k - 8 * s
                        if j < 2:
                            rhs = bl[:, j * SW : (j + 1) * SW]
                        else:
                            o = 2 * SW + TAILOFF[j - 2]
                            rhs = bl[:, o : o + w]
                        nc.tensor.matmul(
                            psums[t][:, SW - w : SW],
                            lhs(k, t),
                            rhs,
                            start=(k == 4 * t),
                            stop=(k == kmax),
                        )
                    w0 = SW - _width(s, 4 * t)
                    ot = opool.tile([P, SW], D, tag="o", name="ot")
                    nc.vector.tensor_copy(ot[:, w0:SW], psums[t][:, w0:SW])
                    nc.scalar.dma_start(
                        Cm[P * t : P * (t + 1), SW * s + w0 : SW * (s + 1)],
                        ot[:, w0:SW],
                    )
    nc.compile()
    return nc


def _get_nc():
    if "nc" not in _cache:
        _cache["nc"] = _build()
    return _cache["nc"]


def _np_bf16():
    import ml_dtypes

    return np.dtype(ml_dtypes.bfloat16)


def _make_in_maps(A, B):
    A = np.asarray(A, dtype=np.float32)
    B = np.asarray(B, dtype=np.float32)
    Au = np.triu(A)
    Bu = np.triu(B)
    bf16 = _np_bf16()

    Au4 = Au.reshape(KT, P, KT, P)  # [mt, ml, kt, p]
    Bu4 = Bu.reshape(KT, P, KT, P)  # [kt, p, nt, q]

    # A payload depends only on r; B payload only on c
    A_r = []
    for r in range(RG):
        ATd = np.empty((P, ATOT, P), dtype=bf16)
        for k in range(KT):
            for t in range(_NSK[k]):
                # lhsT tile: [p, ml] = Au[(4t+r)*128+ml, k*128+p]
                ATd[:, OFFK[k] + t, :] = Au4[4 * t + r, :, k, :].T
        A_r.append(ATd)

    B_c = []
    for c in range(CG):
        segs = []
        for s in range(NSUP):
            nt0 = 8 * s + c
            # full-width region: k < 8s+2, all 4 owned col-tiles of the super
            full = Bu4[: 8 * s + 2, :, nt0 : nt0 + 8 : 2, :]  # [K, p, 4, q]
            segs.append(
                np.ascontiguousarray(full.transpose(1, 0, 2, 3)).reshape(
                    P, (8 * s + 2) * SW
                )
            )
            for j, w in enumerate(TAILW):
                k = 8 * s + 2 + j
                i0 = 4 - w // P
                tail = Bu4[k, :, nt0 + 2 * i0 : nt0 + 8 : 2, :]  # [p, 4-i0, q]
                segs.append(np.ascontiguousarray(tail).reshape(P, w))
        B_c.append(np.concatenate(segs, axis=1).astype(bf16))

    in_maps = []
    for j in range(NCORES):
        r, c = j // CG, j % CG
        in_maps.append({"AT": A_r[r], "B": B_c[c]})
    return in_maps


def kernel(A, B):
    from concourse.bass_utils import run_bass_kernel_spmd

    in_maps = _make_in_maps(A, B)
    nc = _get_nc()
    res = run_bass_kernel_spmd(nc, in_maps, core_ids=list(range(NCORES)))

    C4 = np.zeros((KT, P, KT, P), dtype=np.float32)
    for j in range(NCORES):
        r, c = j // CG, j % CG
        Cj = np.asarray(res.results[j]["C"]).astype(np.float32)
        # rows: slot-major (t -> row-tile 4t+r); cols: (s, i) -> col-tile
        # 8s+2i+c, which is exactly c::2 in ascending order
        C4[r::RG, :, c::CG, :] = Cj.reshape(NSLOT, P, KT // CG, P)
    C = C4.reshape(N, N)
    # below-diagonal tiles the program never stores are uninitialized; the
    # triu kills them (diag-tile interiors are exact zeros from the masking)
    return np.triu(C)


# revision 8
# speedup vs baseline: 1.3888x; 1.0193x over previous
"""Trainium2 Bass kernel for C = triu(triu(A) @ triu(B)), N=4096, fp32.

Math: the product of upper-triangular matrices is upper-triangular, so with
host-side triu masking of A and B the kernel needs no output masking: output
tile (m, n) (128x128 tile indices) only gets contributions from k in [m, n].

Sharding (8 cores, SPMD, one NEFF): 2D grid, 4 row-groups x 2 col-groups.
Core j = (r, c) = (j // 2, j % 2) owns row-tiles {m : m % 4 == r} (8 tiles,
1024 rows) and col-tiles {n : n % 2 == c} (16 tiles, 2048 cols). Versus 1D
row-sharding this (a) cuts per-core HBM traffic ~28MB -> ~16MB because B is
replicated to 4 cores instead of 8, and (b) cuts identical-program masking
waste because the row spread within a slot is 4 instead of 8. All cores run
the identical program; where a core's triangle is smaller than the program's
k-range, the host-side triu masking makes those matmuls accumulate zeros.

Per-core layout: owned cols are packed into 4 local supers of 512 (col-tiles
n = 8s+2i+c, i=0..3, ascending). For super s the program needs k <= 8s+7 and
row slots t with 4t <= 8s+7 (nslots = 2s+2 <= 8 = PSUM bank count).

DMA: the 16 DMA engines are one shared pool and pull per-descriptor, so
streams compete by descriptor size; all loads therefore go on a single
HW-DGE queue (sync) in global need-order, interleaving A chunks (k-major
packed triu, 8 k-tiles per chunk) with B chunks (per super: s full-width
[P,8,512] chunks + one last tile holding k=8s,8s+1 plus the 6 narrowing
tails). Everything stays SBUF-resident; each HBM byte is read once.

Compute: per super, phase 1 runs k-major over the full-width chunks (matches
the load order); phase 2 runs t-major over the last 8 ks so slots finish
staggered - each slot's psum->sbuf bf16 copy (DVE; the ACT copy path is ~9x
slower) and C store (scalar HW-DGE queue) issue immediately, pipelining the
drain instead of serializing ~8 copies after the final matmul. C is written
bf16 and upcast on the host.
"""

import sys

for _p in ("/opt/trn_rl_repo", "/root/.axon_site/_ro/trn_rl_repo"):
    if _p not in sys.path:
        sys.path.insert(0, _p)

import numpy as np

N = 4096
P = 128
KT = N // P  # 32 k-tiles
NCORES = 8
RG = 4  # row groups
CG = 2  # col groups
NSLOT = KT // RG  # 8 row-tiles per core
NSUP = 4  # local col supers per core
SW = 512  # super width (cols)

# widths of the tail matmuls (k = 8s+2+j): union over c of cols >= k
TAILW = [384, 384, 256, 256, 128, 128]
TAILOFF = [0, 384, 768, 1024, 1280, 1408]
TAILSZ = 1536

# nslots at k: slots t with 4t <= k (capped at 8)
_NSK = [min(k // 4 + 1, NSLOT) for k in range(KT)]
# k-major packed A offsets (in 128-wide units)
OFFK = [0] * (KT + 1)
for _k in range(KT):
    OFFK[_k + 1] = OFFK[_k] + _NSK[_k]
ATOT = OFFK[KT]  # 144

# A load chunk boundaries (k-tiles): first chunks finer so the first
# matmuls' inputs arrive ASAP
ACHB = [0, 4, 8, 16, 24, 32]

# B per super: s full-width chunks of [P, 8, SW] (k in [0, 8s)), then one
# "last" tile [P, 2*SW + TAILSZ] holding k = 8s, 8s+1 full width + 6 tails
LASTSZ = 2 * SW + TAILSZ  # 2560
SZSUP = [8 * s * SW + LASTSZ for s in range(NSUP)]
BOFF = [0] * (NSUP + 1)
for _s in range(NSUP):
    BOFF[_s + 1] = BOFF[_s] + SZSUP[_s]
BTOT = BOFF[NSUP]  # 34816


def _width(s, k):
    """matmul free width at (super s, k): cols the program still covers."""
    if k < 8 * s + 2:
        return SW
    return TAILW[k - (8 * s + 2)]


_cache = {}


def _build():
    import concourse.bacc as bacc
    import concourse.mybir as mybir
    import concourse.tile as tile

    D = mybir.dt.bfloat16
    f32 = mybir.dt.float32

    nc = bacc.Bacc(None, target_bir_lowering=False)
    # A packed k-major, lhsT layout: AT[p, OFFK[k]+t, ml] = Au[(4t+r)*128+ml, k*128+p]
    Am = nc.dram_tensor("AT", [P, ATOT, P], D, kind="ExternalInput")
    # B packed per super (full chunks + last tile), per-partition contiguous
    Bm = nc.dram_tensor("B", [P, BTOT], D, kind="ExternalInput")
    # C rows: slot-major (8*128), cols: super-major (4*512), bf16
    Cm = nc.dram_tensor("C", [NSLOT * P, NSUP * SW], D, kind="ExternalOutput")

    with tile.TileContext(nc) as tc:
        with (
            tc.tile_pool(name="a", bufs=1) as apool,
            tc.tile_pool(name="b", bufs=1) as bpool,
            tc.tile_pool(name="o", bufs=8) as opool,
            tc.tile_pool(name="ps", bufs=8, space="PSUM") as pspool,
        ):
            a_tiles = [None] * (len(ACHB) - 1)
            bf8_tiles = [[None] * s for s in range(NSUP)]
            blast_tiles = [None] * NSUP  # s=0 handled by b0k01/b0t below

            def load_a(g):
                w = OFFK[ACHB[g + 1]] - OFFK[ACHB[g]]
                ag = apool.tile([P, w, P], D, tag=f"a{g}", name="ag")
                nc.sync.dma_start(ag[:], Am[:, OFFK[ACHB[g]] : OFFK[ACHB[g + 1]], :])
                a_tiles[g] = ag

            def load_bf8(s, i):
                bt = bpool.tile([P, 8, SW], D, tag=f"b{s}f{i}", name="bt")
                o = BOFF[s] + 8 * i * SW
                nc.sync.dma_start(bt[:], Bm[:, o : o + 8 * SW])
                bf8_tiles[s][i] = bt

            def load_blast(s, split=False):
                if split:
                    # first loads: split so the first matmul starts sooner
                    b01 = bpool.tile([P, 2 * SW], D, tag=f"b{s}l01", name="b01")
                    nc.sync.dma_start(b01[:], Bm[:, BOFF[s] : BOFF[s] + 2 * SW])
                    return b01
                bt = bpool.tile([P, LASTSZ], D, tag=f"b{s}l", name="bl")
                o = BOFF[s] + 8 * s * SW
                nc.sync.dma_start(bt[:], Bm[:, o : o + LASTSZ])
                blast_tiles[s] = bt

            # single queue, global need-order: nothing starves anything
            b0k01 = load_blast(0, split=True)
            load_a(0)  # k0-3
            b0t = bpool.tile([P, TAILSZ], D, tag="b0t", name="b0t")
            nc.sync.dma_start(
                b0t[:], Bm[:, BOFF[0] + 2 * SW : BOFF[0] + 2 * SW + TAILSZ]
            )
            load_a(1)  # k4-7
            load_bf8(1, 0)
            load_a(2)  # k8-15
            load_blast(1)
            load_bf8(2, 0)
            load_bf8(2, 1)
            load_a(3)  # k16-23
            load_blast(2)
            load_bf8(3, 0)
            load_bf8(3, 1)
            load_bf8(3, 2)
            load_a(4)  # k24-31
            load_blast(3)

            # warm-up: the first real matmul can't start until the first
            # loads land (~5us); spend that window running throwaway matmuls
            # so the PE p-state ramp (0.65 -> 1.2 -> 2.4 GHz over ~3us of
            # continuous busy) completes before real work arrives. They
            # target super-0's psum bank as standalone start/stop groups;
            # the real chain re-starts the bank so WAW order is enough.
            warm = apool.tile([P, SW], D, tag="warm", name="warm")
            nc.gpsimd.memset(warm[:], 0)
            ps0 = [
                pspool.tile([P, SW], f32, tag="ps", name="ps") for _ in range(2)
            ]
            for _ in range(12):
                nc.tensor.matmul(
                    ps0[0][:], warm[:, :P], warm[:], start=True, stop=True
                )

            from bisect import bisect_right

            def lhs(k, t):
                g = bisect_right(ACHB, k) - 1
                return a_tiles[g][:, OFFK[k] - OFFK[ACHB[g]] + t, :]

            for s in range(NSUP):
                kmax = 8 * s + 7
                ns = 2 * s + 2
                psums = ps0 if s == 0 else [
                    pspool.tile([P, SW], f32, tag="ps", name="ps")
                    for _ in range(ns)
                ]

                # phase 1: k-major over the full-width chunks (load order)
                for k in range(8 * s):
                    rhs = bf8_tiles[s][k // 8][:, k % 8, :]
                    for t in range(k // 4 + 1):
                        nc.tensor.matmul(
                            psums[t][:],
                            lhs(k, t),
                            rhs,
                            start=(k == 4 * t),
                            stop=False,
                        )
                # phase 2: t-major over the last 8 ks; slots finish staggered
                # so the copy+store drain pipelines with remaining matmuls
                bl = blast_tiles[s]
                for t in range(ns):
                    for k in range(max(4 * t, 8 * s), 8 * s + 8):
                        w = _width(s, k)
                        j = k - 8 * s
                        if s == 0:
                            rhs = (
                                b0k01[:, j * SW : (j + 1) * SW]
                                if j < 2
                                else b0t[:, TAILOFF[j - 2] : TAILOFF[j - 2] + w]
                            )
                        elif j < 2:
                            rhs = bl[:, j * SW : (j + 1) * SW]
                        else:
                            o = 2 * SW + TAILOFF[j - 2]
                            rhs = bl[:, o : o + w]
                        nc.tensor.matmul(
                            psums[t][:, SW - w : SW],
                            lhs(k, t),
                            rhs,
                            start=(k == 4 * t),
                            stop=(k == kmax),
                        )
                    w0 = SW - _width(s, 4 * t)
                    ot = opool.tile([P, SW], D, tag="o", name="ot")
                    nc.vector.tensor_copy(ot[:, w0:SW], psums[t][:, w0:SW])
                    nc.scalar.dma_start(
                        Cm[P * t : P * (t + 1), SW * s + w0 : SW * (s + 1)],
                        ot[:, w0:SW],
                    )
    nc.compile()
    return nc


def _get_nc():
    if "nc" not in _cache:
        _cache["nc"] = _build()
    return _cache["nc"]


def _np_bf16():
    import ml_dtypes

    return np.dtype(ml_dtypes.bfloat16)


def _make_in_maps(A, B):
    A = np.asarray(A, dtype=np.float32)
    B = np.asarray(B, dtype=np.float32)
    Au = np.triu(A)
    Bu = np.triu(B)
    bf16 = _np_bf16()

    Au4 = Au.reshape(KT, P, KT, P)  # [mt, ml, kt, p]
    Bu4 = Bu.reshape(KT, P, KT, P)  # [kt, p, nt, q]

    # A payload depends only on r; B payload only on c
    A_r = []
    for r in range(RG):
        ATd = np.empty((P, ATOT, P), dtype=bf16)
        for k in range(KT):
            for t in range(_NSK[k]):
                # lhsT tile: [p, ml] = Au[(4t+r)*128+ml, k*128+p]
                ATd[:, OFFK[k] + t, :] = Au4[4 * t + r, :, k, :].T
        A_r.append(ATd)

    B_c = []
    for c in range(CG):
        segs = []
        for s in range(NSUP):
            nt0 = 8 * s + c
            # full-width region: k < 8s+2, all 4 owned col-tiles of the super
            full = Bu4[: 8 * s + 2, :, nt0 : nt0 + 8 : 2, :]  # [K, p, 4, q]
            segs.append(
                np.ascontiguousarray(full.transpose(1, 0, 2, 3)).reshape(
                    P, (8 * s + 2) * SW
                )
            )
            for j, w in enumerate(TAILW):
                k = 8 * s + 2 + j
                i0 = 4 - w // P
                tail = Bu4[k, :, nt0 + 2 * i0 : nt0 + 8 : 2, :]  # [p, 4-i0, q]
                segs.append(np.ascontiguousarray(tail).reshape(P, w))
        B_c.append(np.concatenate(segs, axis=1).astype(bf16))

    in_maps = []
    for j in range(NCORES):
        r, c = j // CG, j % CG
        in_maps.append({"AT": A_r[r], "B": B_c[c]})
    return in_maps


def kernel(A, B):
    from concourse.bass_utils import run_bass_kernel_spmd

    in_maps = _make_in_maps(A, B)
    nc = _get_nc()
    res = run_bass_kernel_spmd(nc, in_maps, core_ids=list(range(NCORES)))

    C4 = np.zeros((KT, P, KT, P), dtype=np.float32)
    for j in range(NCORES):
        r, c = j // CG, j % CG
        Cj = np.asarray(res.results[j]["C"]).astype(np.float32)
        # rows: slot-major (t -> row-tile 4t+r); cols: (s, i) -> col-tile
        # 8s+2i+c, which is exactly c::2 in ascending order
        C4[r::RG, :, c::CG, :] = Cj.reshape(NSLOT, P, KT // CG, P)
    C = C4.reshape(N, N)
    # below-diagonal tiles the program never stores are uninitialized; the
    # triu kills them (diag-tile interiors are exact zeros from the masking)
    return np.triu(C)
